# revision 13
# baseline (speedup 1.0000x reference)
"""Pairwise Euclidean distance matrix on 8 TRN2 NeuronCores (Bass/Tile).

out[i, j] = ||x[j] - x[i]||_2 for x [4096, 512] fp32.

Distance symmetry: out = out.T, so only ~half the blocks are computed.
Half-ring decomposition: core c owns query (column) block c and computes
it against key (row) blocks {c, c+1, .., c+4 mod 8} — 5 of 8 blocks,
perfectly balanced and SPMD-uniform. Blocks at ring distance 1..3 are
mirrored into their transposed position on the host during unsharding;
distance 0/4 positions are covered directly.

d2 = sq[i] + sq[j] - 2*x[i].x[j] via PE matmuls. The Gram part runs as a
split-bf16 product (x = hi + lo in bf16; hi.hi + hi.lo + lo.hi
accumulated into the same fp32 PSUM tile) — fp32-class accuracy at bf16
speed. Queries are pre-scaled by -2 on host (exact in bf16), so
PSUM = d2 - sq_m - sq_n; epilogue: DVE adds sq_m (replicated over
partitions), ACT computes Sqrt(x + sq_n) with sq_n as per-partition
bias. The diagonal (d2 == 0 exactly) is zeroed on host.
"""

import numpy as np
import ml_dtypes

import concourse.bass as bass
import concourse.bacc as bacc
import concourse.tile as tile
from concourse.bass_utils import run_bass_kernel_spmd

mybir = bass.mybir

N = 4096          # number of points
D = 512           # feature dim
NCORES = 8
QB = N // NCORES  # 512 queries per core
KT = D // 128     # 4 contraction tiles
RB = 5            # row blocks per core (half-ring)
NT = RB * QB // 128   # 20 key tiles of 128 per core
KEYS = RB * QB        # 2560 keys per core
CG = [512, 1024, 1024]  # key column grouping for DMA staging

_BF16 = mybir.dt.bfloat16
_F32 = mybir.dt.float32

_nc_cache = {}


def _build():
    if "nc" in _nc_cache:
        return _nc_cache["nc"]
    nc = bacc.Bacc(
        "TRN2", target_bir_lowering=False, debug=False, num_swdge_queues=4
    )

    xh = nc.dram_tensor("xh", [D, KEYS], _BF16, kind="ExternalInput")
    xl = nc.dram_tensor("xl", [D, KEYS], _BF16, kind="ExternalInput")
    # hi and lo query halves packed side by side: one DMA brings both
    q = nc.dram_tensor("q", [D, 2 * QB], _BF16, kind="ExternalInput")
    sqn = nc.dram_tensor("sqn", [128, NT], _F32, kind="ExternalInput")
    sqm = nc.dram_tensor("sqm", [128, QB], _F32, kind="ExternalInput")
    out = nc.dram_tensor("out", [KEYS, QB], _F32, kind="ExternalOutput")

    with tile.TileContext(nc) as tc:
        with (
            tc.tile_pool(name="xd", bufs=1) as xd,
            tc.tile_pool(name="op", bufs=4) as op,
            tc.tile_pool(name="ps", bufs=8, space="PSUM") as pp,
        ):
            # Queries first (small, needed by every matmul), then key
            # column-groups in consumption order so tile j's weights land
            # well before the PE reaches them.
            # DMA triggers cost ~640ns each and serialize per engine, so
            # spread them: queries on sync, sq tables on scalar, keys on
            # gpsimd — the first matmul's inputs all trigger in the first
            # ~2.5us instead of queueing behind 10+ triggers.
            t_q = []
            for k in range(KT):
                t = xd.tile([128, 2 * QB], _BF16, tag=f"q{k}", name=f"q{k}")
                nc.sync.dma_start(t[:], q.ap()[k * 128 : (k + 1) * 128, :])
                t_q.append(t)
            t_qh = [t[:, 0:QB] for t in t_q]
            t_ql = [t[:, QB : 2 * QB] for t in t_q]

            t_sqn = xd.tile([128, NT], _F32, tag="sqn", name="sqn")
            nc.scalar.dma_start(t_sqn[:], sqn.ap())
            t_sqm = xd.tile([128, QB], _F32, tag="sqm", name="sqm")
            nc.scalar.dma_start(t_sqm[:], sqm.ap())

            # key tiles: [128, cols] per (group, k, hi/lo). The very first
            # tiles gate the first matmul, so their triggers go to
            # otherwise-idle engines for parallel issue.
            t_keys = {}  # (g, k, part) -> (tile, col0)
            col0 = 0
            for g, cols in enumerate(CG):
                for part, src in (("h", xh), ("l", xl)):
                    for k in range(KT):
                        t = xd.tile(
                            [128, cols], _BF16,
                            tag=f"x{part}{g}_{k}", name=f"x{part}{g}_{k}",
                        )
                        nc.gpsimd.dma_start(
                            t[:],
                            src.ap()[k * 128 : (k + 1) * 128, col0 : col0 + cols],
                        )
                        t_keys[(g, k, part)] = (t, col0)
                col0 += cols

            def wslice(g, k, part, j):
                t, c0 = t_keys[(g, k, part)]
                lo = j * 128 - c0
                return t[:, lo : lo + 128]

            sqrt = mybir.ActivationFunctionType.Sqrt
            bounds = np.cumsum([0] + CG)
            for j in range(NT):
                g = int(np.searchsorted(bounds, j * 128, side="right") - 1)
                p = pp.tile([128, QB], _F32, tag="ps", name=f"ps{j}")
                # hh and hl share the same stationary weight (xh slice), so
                # pair them per k; lo.hi weights follow.
                plan = [("h", k, tq) for k in range(KT) for tq in (t_qh, t_ql)]
                plan += [("l", k, t_qh) for k in range(KT)]
                for mmi, (part, k, tq) in enumerate(plan):
                    nc.tensor.matmul(
                        p[:],
                        wslice(g, k, part, j),
                        tq[k][:],
                        start=(mmi == 0),
                        stop=(mmi == len(plan) - 1),
                    )
                o = op.tile([128, QB], _F32, tag="o", name=f"o{j}")
                # Last tiles: halve the epilogue so add/sqrt/store pipeline
                # instead of serializing on the kernel tail.
                nh = 2 if j >= NT - 2 else 1
                for h in range(nh):
                    sl = slice(h * QB // nh, (h + 1) * QB // nh)
                    nc.vector.tensor_add(o[:, sl], p[:, sl], t_sqm[:, sl])
                    nc.scalar.activation(
                        o[:, sl], o[:, sl], sqrt,
                        bias=t_sqn[:, j : j + 1], scale=1.0,
                    )
                    nc.sync.dma_start(
                        out.ap()[j * 128 : (j + 1) * 128, sl], o[:, sl]
                    )

    nc.compile()
    _nc_cache["nc"] = nc
    return nc


def _ring(c):
    return [(c + t) % NCORES for t in range(RB)]


def _prep_inputs(x: np.ndarray):
    x = np.ascontiguousarray(x, dtype=np.float32)
    xh16 = x.astype(ml_dtypes.bfloat16)
    xh32 = xh16.astype(np.float32)
    xl16 = (x - xh32).astype(ml_dtypes.bfloat16)
    xl32 = xl16.astype(np.float32)

    xe = xh32.astype(np.float64) + xl32.astype(np.float64)
    sq = np.einsum("nd,nd->n", xe, xe)

    xhT = np.ascontiguousarray(xh16.T)  # [D, N]
    xlT = np.ascontiguousarray(xl16.T)

    in_maps = []
    for c in range(NCORES):
        r0 = c * QB
        rows = _ring(c)
        keycols = np.concatenate([np.arange(r * QB, (r + 1) * QB) for r in rows])
        sq_keys = sq[keycols].astype(np.float32)
        in_maps.append(
            {
                "xh": np.ascontiguousarray(xhT[:, keycols]),
                "xl": np.ascontiguousarray(xlT[:, keycols]),
                "q": np.ascontiguousarray(
                    np.concatenate(
                        [
                            (-2.0 * xh32[r0 : r0 + QB]).astype(ml_dtypes.bfloat16).T,
                            (-2.0 * xl32[r0 : r0 + QB]).astype(ml_dtypes.bfloat16).T,
                        ],
                        axis=1,
                    )
                ),
                "sqn": np.ascontiguousarray(sq_keys.reshape(NT, 128).T),
                "sqm": np.ascontiguousarray(
                    np.broadcast_to(sq[r0 : r0 + QB].astype(np.float32), (128, QB))
                ),
            }
        )
    return in_maps


def run(x: np.ndarray, trace: bool = False, tmpdir: str | None = None):
    nc = _build()
    in_maps = _prep_inputs(x)
    res = run_bass_kernel_spmd(
        nc, in_maps, list(range(NCORES)), trace=trace, tmpdir=tmpdir
    )
    full = np.empty((N, N), dtype=np.float32)
    for c in range(NCORES):
        blk = res.results[c]["out"]  # [KEYS, QB]
        for t, r in enumerate(_ring(c)):
            b = blk[t * QB : (t + 1) * QB, :]  # rows r*QB.., cols c*QB..
            full[r * QB : (r + 1) * QB, c * QB : (c + 1) * QB] = b
            if t in (1, 2, 3):  # ring distance 1..3: mirror transpose
                full[c * QB : (c + 1) * QB, r * QB : (r + 1) * QB] = b.T
    np.fill_diagonal(full, 0.0)
    return full, res


def kernel(x: np.ndarray) -> np.ndarray:
    out, _ = run(x, trace=False)
    return out
